# revision 22
# baseline (speedup 1.0000x reference)
"""MLA attention (B=1, S=2048, D=1024, H=16, HD=64, LAT=256) on 8 TRN2 cores.

Sharding: tensor-parallel over heads. Each core owns 2 heads (a 128-wide
slice of the q/k/v up-projections and of Wo's rows), replicates the latent
down-projection chain, runs causal attention for its heads, and produces a
partial output  attn_out_c @ Wo[128c:128(c+1), :].  The host sums the 8
partials (row-parallel o_proj unshard).

Self-contained: hardcodes all shapes; builds the Bass program once and runs
it via bass_utils.run_bass_kernel_spmd on cores 0-7.
"""

import os
import sys

for _p in ("/opt/trn_rl_repo",):
    if _p not in sys.path:
        sys.path.append(_p)

import numpy as np

import concourse.bass as bass  # noqa: F401  (registers engine classes)
import concourse.mybir as mybir
import concourse.tile as tile
from concourse import bacc
from concourse.bass_utils import run_bass_kernel_spmd
from concourse import library_config

F32 = mybir.dt.float32
F32R = mybir.dt.float32r

B, S, D = 1, 2048, 1024
H, HD, LAT, KV = 16, 64, 256, 512
N_CORES = 8
HPC = H // N_CORES          # heads per core = 2
FPC = HPC * HD              # features per core = 128
CH = 512                    # sequence chunk
N_CH = S // CH              # 4
N_ST = S // 128             # 16 sequence tiles
N_DT = D // 128             # 8
N_FT = KV // 128            # 4
N_LT = LAT // 128           # 2
ROPE_BASE = 10000.0
SCALE = 1.0 / np.sqrt(HD)

_CACHE = {}


def _rope_tables():
    inv_freq = (1.0 / (ROPE_BASE ** (np.arange(0, HD, 2, dtype=np.float32) / HD))).astype(np.float32)
    pos = np.arange(S, dtype=np.float32)
    freqs = pos[:, None] * inv_freq[None, :]            # [S, 32]
    emb = np.concatenate([freqs, freqs], axis=-1)       # [S, 64]
    cosT = np.cos(emb).T.astype(np.float32)             # [64, S]
    sinT = np.sin(emb).T.astype(np.float32)
    cos2 = np.tile(cosT, (HPC, 1))                      # [128, S]
    sin2 = np.tile(sinT, (HPC, 1))
    return np.ascontiguousarray(cos2), np.ascontiguousarray(sin2)


def _mask_strip():
    # strip[kp, j] = 1.0 iff (j - 384) >= kp ;  block r uses cols [512-128(r+1), 512)
    kp = np.arange(128)[:, None]
    j = np.arange(512)[None, :]
    return ((j - 384) >= kp).astype(np.float32)


def _R(ap):
    return ap.bitcast(F32R)


def _build_nc():
    nc = bacc.Bacc("TRN2", target_bir_lowering=False, debug=False, num_devices=N_CORES)

    x_d = nc.dram_tensor("x", [S, D], F32, kind="ExternalInput").ap()
    wq_d = nc.dram_tensor("wq", [D, FPC], F32, kind="ExternalInput").ap()
    wk_d = nc.dram_tensor("wk", [D, KV], F32, kind="ExternalInput").ap()
    wkd_d = nc.dram_tensor("wkd", [KV, LAT], F32, kind="ExternalInput").ap()
    wv_d = nc.dram_tensor("wv", [D, KV], F32, kind="ExternalInput").ap()
    wvd_d = nc.dram_tensor("wvd", [KV, LAT], F32, kind="ExternalInput").ap()
    wku_d = nc.dram_tensor("wku", [LAT, FPC], F32, kind="ExternalInput").ap()
    wvu_d = nc.dram_tensor("wvu", [LAT, FPC], F32, kind="ExternalInput").ap()
    wo_d = nc.dram_tensor("wo", [FPC, D], F32, kind="ExternalInput").ap()
    wqr_d = nc.dram_tensor("wqr", [D, FPC], F32, kind="ExternalInput").ap()
    wkur_d = nc.dram_tensor("wkur", [LAT, FPC], F32, kind="ExternalInput").ap()
    cos_d = nc.dram_tensor("cos2", [FPC, S], F32, kind="ExternalInput").ap()
    sin_d = nc.dram_tensor("sin2", [FPC, S], F32, kind="ExternalInput").ap()
    msk_d = nc.dram_tensor("mask", [128, 512], F32, kind="ExternalInput").ap()
    idn_d = nc.dram_tensor("ident", [128, 128], F32, kind="ExternalInput").ap()
    ones_d = nc.dram_tensor("ones16", [128, N_ST], F32, kind="ExternalInput").ap()
    out_d = nc.dram_tensor("out", [S, D], F32, kind="ExternalOutput").ap()

    from contextlib import ExitStack
    with tile.TileContext(nc) as tc, ExitStack() as stk:
        nc.gpsimd.load_library(library_config.attn)
        consts = stk.enter_context(tc.tile_pool(name="consts", bufs=1))
        persist = stk.enter_context(tc.tile_pool(name="persist", bufs=1))
        xnat_p = stk.enter_context(tc.tile_pool(name="xnat", bufs=4))
        xT_p = stk.enter_context(tc.tile_pool(name="xT", bufs=1))
        kv_p = stk.enter_context(tc.tile_pool(name="kv", bufs=2))
        lat_p = stk.enter_context(tc.tile_pool(name="lat", bufs=3))
        vtmp_p = stk.enter_context(tc.tile_pool(name="vtmp", bufs=2))
        rope_p = stk.enter_context(tc.tile_pool(name="rope", bufs=2))
        exp_p = stk.enter_context(tc.tile_pool(name="expp", bufs=3))
        small_p = stk.enter_context(tc.tile_pool(name="small", bufs=2))
        ob_p = stk.enter_context(tc.tile_pool(name="ob", bufs=3))
        tr_ps = stk.enter_context(tc.tile_pool(name="trps", bufs=2, space="PSUM"))
        pp_ps = stk.enter_context(tc.tile_pool(name="pp", bufs=2, space="PSUM"))
        sc_ps = stk.enter_context(tc.tile_pool(name="scps", bufs=2, space="PSUM"))
        av_ps = stk.enter_context(tc.tile_pool(name="avps", bufs=2, space="PSUM"))

        # ---- constants ----
        wk_sb = consts.tile([128, N_DT, KV], F32)
        wv_sb = consts.tile([128, N_DT, KV], F32)
        wq_sb = consts.tile([128, N_DT, FPC], F32)
        wkd_sb = consts.tile([128, N_FT, LAT], F32)
        wvd_sb = consts.tile([128, N_FT, LAT], F32)
        wqr_sb = consts.tile([128, N_DT, FPC], F32)
        wkur_sb = consts.tile([128, N_LT, FPC], F32)
        wku_sb = consts.tile([128, N_LT, FPC], F32)
        wvu_sb = consts.tile([128, N_LT, FPC], F32)
        wo_sb = consts.tile([128, D], F32)
        cos_sb = consts.tile([128, S], F32)
        sin_sb = consts.tile([128, S], F32)
        msk_sb = consts.tile([128, 512], F32)
        idn_sb = consts.tile([128, 128], F32)
        for dt in range(N_DT):
            nc.sync.dma_start(_R(wk_sb[:, dt, :]), _R(wk_d[dt * 128:(dt + 1) * 128, :]))
            nc.sync.dma_start(_R(wv_sb[:, dt, :]), _R(wv_d[dt * 128:(dt + 1) * 128, :]))
            nc.sync.dma_start(_R(wq_sb[:, dt, :]), _R(wq_d[dt * 128:(dt + 1) * 128, :]))
            nc.sync.dma_start(_R(wqr_sb[:, dt, :]), _R(wqr_d[dt * 128:(dt + 1) * 128, :]))
        for ft in range(N_FT):
            nc.sync.dma_start(_R(wkd_sb[:, ft, :]), _R(wkd_d[ft * 128:(ft + 1) * 128, :]))
            nc.sync.dma_start(_R(wvd_sb[:, ft, :]), _R(wvd_d[ft * 128:(ft + 1) * 128, :]))
        for lt in range(N_LT):
            nc.sync.dma_start(_R(wku_sb[:, lt, :]), _R(wku_d[lt * 128:(lt + 1) * 128, :]))
            nc.sync.dma_start(_R(wkur_sb[:, lt, :]), _R(wkur_d[lt * 128:(lt + 1) * 128, :]))
            nc.sync.dma_start(_R(wvu_sb[:, lt, :]), _R(wvu_d[lt * 128:(lt + 1) * 128, :]))
        nc.sync.dma_start(_R(wo_sb[:]), _R(wo_d[:]))
        nc.sync.dma_start(cos_sb[:], cos_d[:])
        nc.sync.dma_start(sin_sb[:], sin_d[:])
        nc.sync.dma_start(msk_sb[:], msk_d[:])
        nc.sync.dma_start(_R(idn_sb[:]), _R(idn_d[:]))

        _stage0 = os.environ.get("BIS_STAGE", "full")
        if _stage0 == "wload":
            for st in range(4):
                ob0 = ob_p.tile([128, 512], F32, tag="ob")
                nc.vector.tensor_copy(ob0[:], wk_sb[:, 0, :])
                nc.sync.dma_start(out_d[st * 128:(st + 1) * 128, 0:512], ob0[:])
        # ---- persistent activations ----
        qro = persist.tile([128, S], F32)            # roped q^T  (2 heads stacked)
        kro = persist.tile([128, S], F32)            # roped k^T
        vaug = persist.tile([128, N_ST, 2 * (HD + 1)], F32)   # v natural + ones col, per k-tile/head
        o_inT = persist.tile([128, S], F32)          # normalized attn out^T
        nc.sync.dma_start(_R(vaug[:, :, HD:HD + 1]), _R(ones_d[:, :]))
        nc.sync.dma_start(_R(vaug[:, :, 2 * HD + 1:2 * HD + 2]), _R(ones_d[:, :]))

        def rope(ps, psr, out_ap, c0):
            csl = cos_sb[:, c0:c0 + CH]
            ssl = sin_sb[:, c0:c0 + CH]
            tmp = rope_p.tile([128, CH], F32, tag="ropetmp")
            prod = rope_p.tile([128, CH], F32, tag="ropeprod")
            nc.vector.tensor_mul(tmp[:], ps[:], csl)
            nc.vector.tensor_mul(prod[:], psr[:], ssl)
            nc.vector.tensor_add(_R(out_ap[:]), tmp[:], prod[:])

        # ---- per-chunk projection pipeline ----
        _n_ch = {"wload": 0, "xtr": 1, "proj1": 1, "lat1": 1, "xk1": 1, "lat2": 2, "lat3": 3, "q1": 1, "k1": 1, "v1": 1}.get(_stage0, N_CH)
        for ci in range(_n_ch):
            c0 = ci * CH
            xT = xT_p.tile([128, N_DT, CH], F32, tag="xT")
            xns = []
            for st in range(4):
                xn = xnat_p.tile([128, D], F32, tag="xn")
                nc.sync.dma_start(_R(xn[:]), _R(x_d[c0 + st * 128:c0 + (st + 1) * 128, :]))
                xns.append(xn)
            for dt in range(N_DT):
                ps = tr_ps.tile([128, 512], F32, tag="trps")
                for st in range(4):
                    nc.tensor.transpose(
                        _R(ps[:, st * 128:(st + 1) * 128]),
                        _R(xns[st][:, dt * 128:(dt + 1) * 128]),
                        _R(idn_sb[:]),
                    )
                nc.scalar.copy(_R(xT[:, dt, :]), ps[:])

            if _stage0 == "xtr":
                for st in range(4):
                    ob1 = ob_p.tile([128, 512], F32, tag="ob")
                    nc.vector.tensor_copy(ob1[:], xT[:, st, :])
                    nc.sync.dma_start(out_d[st * 128:(st + 1) * 128, 0:512], ob1[:])
                continue
            # xk^T / xv^T then latents
            for which in range(1 if _stage0 == "xk1" else 2):
                w_sb = wk_sb if which == 0 else wv_sb
                wd_sb = wkd_sb if which == 0 else wvd_sb
                xw = kv_p.tile([128, N_FT, CH], F32, tag="xkv")
                for ft in range(N_FT):
                    ps = pp_ps.tile([128, CH], F32, tag="pp")
                    for dt in range(N_DT):
                        nc.tensor.matmul(
                            ps[:], _R(w_sb[:, dt, ft * 128:(ft + 1) * 128]), _R(xT[:, dt, :]),
                            start=(dt == 0), stop=(dt == N_DT - 1),
                        )
                    nc.vector.tensor_copy(_R(xw[:, ft, :]), ps[:])
                latc = lat_p.tile([128, N_LT, CH], F32, tag="lat")
                for lt in range(0 if _stage0 == "xk1" else N_LT):
                    ps = pp_ps.tile([128, CH], F32, tag="pp")
                    for ft in range(N_FT):
                        nc.tensor.matmul(
                            ps[:], _R(wd_sb[:, ft, lt * 128:(lt + 1) * 128]), _R(xw[:, ft, :]),
                            start=(ft == 0), stop=(ft == N_FT - 1),
                        )
                    nc.vector.tensor_copy(_R(latc[:, lt, :]), ps[:])
                if which == 0:
                    latk = latc
                else:
                    latv = latc
            if _stage0 == "xk1":
                continue

            if _stage0 in ("lat", "lat1", "lat2", "lat3"):
                continue
            # q projection + rope
            ps = pp_ps.tile([128, CH], F32, tag="pp")
            psr = pp_ps.tile([128, CH], F32, tag="pp")
            for dt in range(N_DT):
                nc.tensor.matmul(ps[:], _R(wq_sb[:, dt, :]), _R(xT[:, dt, :]),
                                 start=(dt == 0), stop=(dt == N_DT - 1))
            for dt in range(N_DT):
                nc.tensor.matmul(psr[:], _R(wqr_sb[:, dt, :]), _R(xT[:, dt, :]),
                                 start=(dt == 0), stop=(dt == N_DT - 1))
            rope(ps, psr, qro[:, c0:c0 + CH], c0)
            if _stage0 == "q1":
                continue

            # k up-projection + rope
            ps = pp_ps.tile([128, CH], F32, tag="pp")
            psr = pp_ps.tile([128, CH], F32, tag="pp")
            for lt in range(N_LT):
                nc.tensor.matmul(ps[:], _R(wku_sb[:, lt, :]), _R(latk[:, lt, :]),
                                 start=(lt == 0), stop=(lt == N_LT - 1))
            for lt in range(N_LT):
                nc.tensor.matmul(psr[:], _R(wkur_sb[:, lt, :]), _R(latk[:, lt, :]),
                                 start=(lt == 0), stop=(lt == N_LT - 1))
            rope(ps, psr, kro[:, c0:c0 + CH], c0)
            if _stage0 == "k1":
                continue

            # v up-projection -> natural layout + ones column
            ps = pp_ps.tile([128, CH], F32, tag="pp")
            for lt in range(N_LT):
                nc.tensor.matmul(ps[:], _R(wvu_sb[:, lt, :]), _R(latv[:, lt, :]),
                                 start=(lt == 0), stop=(lt == N_LT - 1))
            vtmp = vtmp_p.tile([128, CH], F32, tag="vtmp")
            nc.scalar.copy(_R(vtmp[:]), ps[:])
            if _stage0 == "v1":
                continue
            tps = tr_ps.tile([128, 512], F32, tag="trps")
            for st in range(4):
                nc.tensor.transpose(
                    _R(tps[:, st * 128:(st + 1) * 128]),
                    _R(vtmp[:, st * 128:(st + 1) * 128]),
                    _R(idn_sb[:]),
                )
            for st in range(4):
                kt = ci * 4 + st
                for h in range(HPC):
                    off = st * 128 + h * HD
                    nc.scalar.copy(_R(vaug[:, kt, h * (HD + 1):h * (HD + 1) + HD]), tps[:, off:off + HD])

        # ---- attention ----
        _stage = os.environ.get("BIS_STAGE", "full")
        if _stage in ("proj", "proj1", "lat", "lat1", "xk1", "lat2", "lat3", "q1", "k1", "v1"):
            for st in range(N_ST):
                ob = ob_p.tile([128, 512], F32, tag="ob")
                nc.vector.tensor_copy(ob[:], cos_sb[:, 0:512])
                nc.sync.dma_start(out_d[st * 128:(st + 1) * 128, 0:512], ob[:])
        for qc in range(N_CH) if _stage in ("attn", "full") else []:
            for h in range(HPC):
                hb = h * HD
                nkt = 4 * (qc + 1)
                av = av_ps.tile([HD + 1, 512], F32, tag="avps")
                for kt in range(nkt):
                    sc = sc_ps.tile([128, 512], F32, tag="scps")
                    nc.tensor.matmul(
                        sc[:], _R(kro[hb:hb + HD, kt * 128:(kt + 1) * 128]),
                        _R(qro[hb:hb + HD, qc * 512:(qc + 1) * 512]),
                        start=True, stop=True,
                    )
                    ex = exp_p.tile([128, 512], F32, tag="exp")
                    nc.scalar.activation(_R(ex[:]), sc[:], mybir.ActivationFunctionType.Exp, scale=float(SCALE))
                    r = kt - 4 * qc
                    if r >= 0:
                        w = 128 * (r + 1)
                        nc.vector.tensor_mul(_R(ex[:, 0:w]), ex[:, 0:w], msk_sb[:, 512 - w:512])
                    nc.tensor.matmul(
                        av[:], _R(vaug[:, kt, h * (HD + 1):(h + 1) * (HD + 1)]), _R(ex[:]),
                        start=(kt == 0), stop=(kt == nkt - 1), skip_group_check=True,
                    )
                onrm = small_p.tile([HD, 512], F32, tag="onrm")
                if os.environ.get("BIS_NONORM"):
                    nc.vector.tensor_copy(onrm[:], av[0:HD, :])
                else:
                    nrm = small_p.tile([128, 512], F32, tag="nrm")
                    nc.vector.reciprocal(nrm[HD:HD + 1, :], av[HD:HD + 1, :])
                    rrow = small_p.tile([1, 512], F32, tag="rrow")
                    nc.sync.dma_start(rrow[:], nrm[HD:HD + 1, :])
                    bcast = small_p.tile([HD, 512], F32, tag="bcast")
                    nc.gpsimd.partition_broadcast(bcast[:], rrow[:])
                    nc.vector.tensor_mul(onrm[:], av[0:HD, :], bcast[:])
                if os.environ.get("BIS_NOSBDMA"):
                    nc.scalar.copy(_R(o_inT[0:HD, qc * 512:(qc + 1) * 512]), onrm[:])
                else:
                    nc.sync.dma_start(_R(o_inT[hb:hb + HD, qc * 512:(qc + 1) * 512]), _R(onrm[:]))

        # ---- partial o-projection ----
        if _stage == "attn":
            for st in range(N_ST):
                ob = ob_p.tile([128, 512], F32, tag="ob")
                nc.vector.tensor_copy(ob[:], o_inT[:, 0:512])
                nc.sync.dma_start(out_d[st * 128:(st + 1) * 128, 0:512], ob[:])
        for st in range(N_ST) if _stage == "full" else []:
            for nck in range(2):
                ps = pp_ps.tile([128, 512], F32, tag="pp")
                nc.tensor.matmul(
                    ps[:], _R(o_inT[:, st * 128:(st + 1) * 128]), _R(wo_sb[:, nck * 512:(nck + 1) * 512]),
                    start=True, stop=True,
                )
                ob = ob_p.tile([128, 512], F32, tag="ob")
                nc.vector.tensor_copy(ob[:], ps[:])
                nc.sync.dma_start(out_d[st * 128:(st + 1) * 128, nck * 512:(nck + 1) * 512], ob[:])

    nc.compile()
    return nc


def _get_nc():
    if "nc" not in _CACHE:
        _CACHE["nc"] = _build_nc()
    return _CACHE["nc"]


def _rot_cols(w):
    """Per 64-col head block: [w1, w2] -> [-w2, w1]  (rotate_half on output features)."""
    w = np.asarray(w, np.float32)
    out = np.empty_like(w)
    for h in range(w.shape[1] // HD):
        b = h * HD
        out[:, b:b + 32] = -w[:, b + 32:b + 64]
        out[:, b + 32:b + 64] = w[:, b:b + 32]
    return np.ascontiguousarray(out)


def make_in_maps(x, Wq, Wk, Wv, Wkd, Wvd, Wku, Wvu, Wo):
    x2 = np.ascontiguousarray(np.asarray(x, dtype=np.float32).reshape(S, D))
    cos2, sin2 = _rope_tables()
    msk = _mask_strip()
    idn = np.eye(128, dtype=np.float32)
    common = {
        "x": x2,
        "wk": np.ascontiguousarray(np.asarray(Wk, np.float32)),
        "wkd": np.ascontiguousarray(np.asarray(Wkd, np.float32)),
        "wv": np.ascontiguousarray(np.asarray(Wv, np.float32)),
        "wvd": np.ascontiguousarray(np.asarray(Wvd, np.float32)),
        "cos2": cos2, "sin2": sin2, "mask": msk, "ident": idn,
        "ones16": np.ones((128, N_ST), np.float32),
    }
    Wq = np.asarray(Wq, np.float32)
    Wku = np.asarray(Wku, np.float32)
    Wvu = np.asarray(Wvu, np.float32)
    Wo = np.asarray(Wo, np.float32)
    in_maps = []
    for c in range(N_CORES):
        sl = slice(c * FPC, (c + 1) * FPC)
        in_maps.append(dict(
            common,
            wq=np.ascontiguousarray(Wq[:, sl]),
            wqr=_rot_cols(Wq[:, sl]),
            wkur=_rot_cols(Wku[:, sl]),
            wku=np.ascontiguousarray(Wku[:, sl]),
            wvu=np.ascontiguousarray(Wvu[:, sl]),
            wo=np.ascontiguousarray(Wo[sl, :]),
        ))
    return in_maps


def kernel(x, Wq, Wk, Wv, Wkd, Wvd, Wku, Wvu, Wo):
    nc = _get_nc()
    in_maps = make_in_maps(x, Wq, Wk, Wv, Wkd, Wvd, Wku, Wvu, Wo)
    res = run_bass_kernel_spmd(nc, in_maps, list(range(N_CORES)))
    acc = res.results[0]["out"].astype(np.float32)
    for c in range(1, N_CORES):
        acc = acc + res.results[c]["out"]
    return acc.reshape(B, S, D)


# revision 23
# speedup vs baseline: 1.1035x; 1.1035x over previous
"""MLA attention (B=1, S=2048, D=1024, H=16, HD=64, LAT=256) on 8 TRN2 cores.

Sharding: tensor-parallel over heads. Each core owns 2 heads (a 128-wide
slice of the q/k/v up-projections and of Wo's rows), replicates the latent
down-projection chain, runs causal attention for its heads, and produces a
partial output  attn_out_c @ Wo[128c:128(c+1), :].  The host sums the 8
partials (row-parallel o_proj unshard).

Self-contained: hardcodes all shapes; builds the Bass program once and runs
it via bass_utils.run_bass_kernel_spmd on cores 0-7.
"""

import os
import sys

for _p in ("/opt/trn_rl_repo",):
    if _p not in sys.path:
        sys.path.append(_p)

import numpy as np

import concourse.bass as bass  # noqa: F401  (registers engine classes)
import concourse.mybir as mybir
import concourse.tile as tile
from concourse import bacc
from concourse.bass_utils import run_bass_kernel_spmd
from concourse import library_config

F32 = mybir.dt.float32
F32R = mybir.dt.float32r

B, S, D = 1, 2048, 1024
H, HD, LAT, KV = 16, 64, 256, 512
N_CORES = 8
HPC = H // N_CORES          # heads per core = 2
FPC = HPC * HD              # features per core = 128
CH = 512                    # sequence chunk
N_CH = S // CH              # 4
N_ST = S // 128             # 16 sequence tiles
N_DT = D // 128             # 8
N_FT = KV // 128            # 4
N_LT = LAT // 128           # 2
ROPE_BASE = 10000.0
SCALE = 1.0 / np.sqrt(HD)

_CACHE = {}


def _rope_tables():
    inv_freq = (1.0 / (ROPE_BASE ** (np.arange(0, HD, 2, dtype=np.float32) / HD))).astype(np.float32)
    pos = np.arange(S, dtype=np.float32)
    freqs = pos[:, None] * inv_freq[None, :]            # [S, 32]
    emb = np.concatenate([freqs, freqs], axis=-1)       # [S, 64]
    cosT = np.cos(emb).T.astype(np.float32)             # [64, S]
    sinT = np.sin(emb).T.astype(np.float32)
    cos2 = np.tile(cosT, (HPC, 1))                      # [128, S]
    sin2 = np.tile(sinT, (HPC, 1))
    return np.ascontiguousarray(cos2), np.ascontiguousarray(sin2)


def _mask_strip():
    # strip[kp, j] = 1.0 iff (j - 384) >= kp ;  block r uses cols [512-128(r+1), 512)
    kp = np.arange(128)[:, None]
    j = np.arange(512)[None, :]
    return ((j - 384) >= kp).astype(np.float32)


def _R(ap):
    return ap.bitcast(F32R)


def _build_nc():
    nc = bacc.Bacc("TRN2", target_bir_lowering=False, debug=False, num_devices=N_CORES)

    x_d = nc.dram_tensor("x", [S, D], F32, kind="ExternalInput").ap()
    wq_d = nc.dram_tensor("wq", [D, FPC], F32, kind="ExternalInput").ap()
    wk_d = nc.dram_tensor("wk", [D, KV], F32, kind="ExternalInput").ap()
    wkd_d = nc.dram_tensor("wkd", [KV, LAT], F32, kind="ExternalInput").ap()
    wv_d = nc.dram_tensor("wv", [D, KV], F32, kind="ExternalInput").ap()
    wvd_d = nc.dram_tensor("wvd", [KV, LAT], F32, kind="ExternalInput").ap()
    wku_d = nc.dram_tensor("wku", [LAT, FPC], F32, kind="ExternalInput").ap()
    wvu_d = nc.dram_tensor("wvu", [LAT, FPC], F32, kind="ExternalInput").ap()
    wo_d = nc.dram_tensor("wo", [FPC, D], F32, kind="ExternalInput").ap()
    wqr_d = nc.dram_tensor("wqr", [D, FPC], F32, kind="ExternalInput").ap()
    wkur_d = nc.dram_tensor("wkur", [LAT, FPC], F32, kind="ExternalInput").ap()
    cos_d = nc.dram_tensor("cos2", [FPC, S], F32, kind="ExternalInput").ap()
    sin_d = nc.dram_tensor("sin2", [FPC, S], F32, kind="ExternalInput").ap()
    msk_d = nc.dram_tensor("mask", [128, 512], F32, kind="ExternalInput").ap()
    idn_d = nc.dram_tensor("ident", [128, 128], F32, kind="ExternalInput").ap()
    ones_d = nc.dram_tensor("ones16", [128, N_ST], F32, kind="ExternalInput").ap()
    out_d = nc.dram_tensor("out", [S, D], F32, kind="ExternalOutput").ap()

    from contextlib import ExitStack
    with tile.TileContext(nc) as tc, ExitStack() as stk:
        nc.gpsimd.load_library(library_config.attn)
        consts = stk.enter_context(tc.tile_pool(name="consts", bufs=1))
        persist = stk.enter_context(tc.tile_pool(name="persist", bufs=1))
        xnat_p = stk.enter_context(tc.tile_pool(name="xnat", bufs=4))
        xT_p = stk.enter_context(tc.tile_pool(name="xT", bufs=1))
        kv_p = stk.enter_context(tc.tile_pool(name="kv", bufs=2))
        lat_p = stk.enter_context(tc.tile_pool(name="lat", bufs=3))
        vtmp_p = stk.enter_context(tc.tile_pool(name="vtmp", bufs=2))
        rope_p = stk.enter_context(tc.tile_pool(name="rope", bufs=2))
        exp_p = stk.enter_context(tc.tile_pool(name="expp", bufs=3))
        small_p = stk.enter_context(tc.tile_pool(name="small", bufs=2))
        ob_p = stk.enter_context(tc.tile_pool(name="ob", bufs=3))
        tr_ps = stk.enter_context(tc.tile_pool(name="trps", bufs=2, space="PSUM"))
        pp_ps = stk.enter_context(tc.tile_pool(name="pp", bufs=2, space="PSUM"))
        sc_ps = stk.enter_context(tc.tile_pool(name="scps", bufs=2, space="PSUM"))
        av_ps = stk.enter_context(tc.tile_pool(name="avps", bufs=2, space="PSUM"))

        # ---- constants ----
        wk_sb = consts.tile([128, N_DT, KV], F32)
        wv_sb = consts.tile([128, N_DT, KV], F32)
        wq_sb = consts.tile([128, N_DT, FPC], F32)
        wkd_sb = consts.tile([128, N_FT, LAT], F32)
        wvd_sb = consts.tile([128, N_FT, LAT], F32)
        wqr_sb = consts.tile([128, N_DT, FPC], F32)
        wkur_sb = consts.tile([128, N_LT, FPC], F32)
        wku_sb = consts.tile([128, N_LT, FPC], F32)
        wvu_sb = consts.tile([128, N_LT, FPC], F32)
        wo_sb = consts.tile([128, D], F32)
        cos_sb = consts.tile([128, S], F32)
        sin_sb = consts.tile([128, S], F32)
        msk_sb = consts.tile([128, 512], F32)
        idn_sb = consts.tile([128, 128], F32)
        for dt in range(N_DT):
            nc.sync.dma_start(_R(wk_sb[:, dt, :]), _R(wk_d[dt * 128:(dt + 1) * 128, :]))
            nc.sync.dma_start(_R(wv_sb[:, dt, :]), _R(wv_d[dt * 128:(dt + 1) * 128, :]))
            nc.sync.dma_start(_R(wq_sb[:, dt, :]), _R(wq_d[dt * 128:(dt + 1) * 128, :]))
            nc.sync.dma_start(_R(wqr_sb[:, dt, :]), _R(wqr_d[dt * 128:(dt + 1) * 128, :]))
        for ft in range(N_FT):
            nc.sync.dma_start(_R(wkd_sb[:, ft, :]), _R(wkd_d[ft * 128:(ft + 1) * 128, :]))
            nc.sync.dma_start(_R(wvd_sb[:, ft, :]), _R(wvd_d[ft * 128:(ft + 1) * 128, :]))
        for lt in range(N_LT):
            nc.sync.dma_start(_R(wku_sb[:, lt, :]), _R(wku_d[lt * 128:(lt + 1) * 128, :]))
            nc.sync.dma_start(_R(wkur_sb[:, lt, :]), _R(wkur_d[lt * 128:(lt + 1) * 128, :]))
            nc.sync.dma_start(_R(wvu_sb[:, lt, :]), _R(wvu_d[lt * 128:(lt + 1) * 128, :]))
        nc.sync.dma_start(_R(wo_sb[:]), _R(wo_d[:]))
        nc.sync.dma_start(cos_sb[:], cos_d[:])
        nc.sync.dma_start(sin_sb[:], sin_d[:])
        nc.sync.dma_start(msk_sb[:], msk_d[:])
        nc.sync.dma_start(_R(idn_sb[:]), _R(idn_d[:]))

        _stage0 = os.environ.get("BIS_STAGE", "full")
        if _stage0 == "wload":
            for st in range(4):
                ob0 = ob_p.tile([128, 512], F32, tag="ob")
                nc.vector.tensor_copy(ob0[:], wk_sb[:, 0, :])
                nc.sync.dma_start(out_d[st * 128:(st + 1) * 128, 0:512], ob0[:])
        # ---- persistent activations ----
        qro = persist.tile([128, S], F32)            # roped q^T  (2 heads stacked)
        kro = persist.tile([128, S], F32)            # roped k^T
        vaug = persist.tile([128, N_ST, 2 * (HD + 1)], F32)   # v natural + ones col, per k-tile/head
        o_inT = persist.tile([128, S], F32)          # normalized attn out^T
        nc.sync.dma_start(_R(vaug[:, :, HD:HD + 1]), _R(ones_d[:, :]))
        nc.sync.dma_start(_R(vaug[:, :, 2 * HD + 1:2 * HD + 2]), _R(ones_d[:, :]))

        _KREPS = int(os.environ.get("KREPS", "1"))

        def rope(ps, psr, out_ap, c0):
            csl = cos_sb[:, c0:c0 + CH]
            ssl = sin_sb[:, c0:c0 + CH]
            tmp = rope_p.tile([128, CH], F32, tag="ropetmp")
            prod = rope_p.tile([128, CH], F32, tag="ropeprod")
            nc.vector.tensor_mul(tmp[:], ps[:], csl)
            nc.vector.tensor_mul(prod[:], psr[:], ssl)
            nc.vector.tensor_add(_R(out_ap[:]), tmp[:], prod[:])

        # ---- per-chunk projection pipeline ----
        _n_ch = {"wload": 0, "xtr": 1, "proj1": 1, "lat1": 1, "xk1": 1, "lat2": 2, "lat3": 3, "q1": 1, "k1": 1, "v1": 1}.get(_stage0, N_CH)
        for _rep, ci in [(r, c) for r in range(_KREPS) for c in range(_n_ch)]:
            c0 = ci * CH
            xT = xT_p.tile([128, N_DT, CH], F32, tag="xT")
            xns = []
            for st in range(4):
                xn = xnat_p.tile([128, D], F32, tag="xn")
                nc.sync.dma_start(_R(xn[:]), _R(x_d[c0 + st * 128:c0 + (st + 1) * 128, :]))
                xns.append(xn)
            for dt in range(N_DT):
                ps = tr_ps.tile([128, 512], F32, tag="trps")
                for st in range(4):
                    nc.tensor.transpose(
                        _R(ps[:, st * 128:(st + 1) * 128]),
                        _R(xns[st][:, dt * 128:(dt + 1) * 128]),
                        _R(idn_sb[:]),
                    )
                nc.scalar.copy(_R(xT[:, dt, :]), ps[:])

            if _stage0 == "xtr":
                for st in range(4):
                    ob1 = ob_p.tile([128, 512], F32, tag="ob")
                    nc.vector.tensor_copy(ob1[:], xT[:, st, :])
                    nc.sync.dma_start(out_d[st * 128:(st + 1) * 128, 0:512], ob1[:])
                continue
            # xk^T / xv^T then latents
            for which in range(1 if _stage0 == "xk1" else 2):
                w_sb = wk_sb if which == 0 else wv_sb
                wd_sb = wkd_sb if which == 0 else wvd_sb
                xw = kv_p.tile([128, N_FT, CH], F32, tag="xkv")
                for ft in range(N_FT):
                    ps = pp_ps.tile([128, CH], F32, tag="pp")
                    for dt in range(N_DT):
                        nc.tensor.matmul(
                            ps[:], _R(w_sb[:, dt, ft * 128:(ft + 1) * 128]), _R(xT[:, dt, :]),
                            start=(dt == 0), stop=(dt == N_DT - 1),
                        )
                    nc.vector.tensor_copy(_R(xw[:, ft, :]), ps[:])
                latc = lat_p.tile([128, N_LT, CH], F32, tag="lat")
                for lt in range(0 if _stage0 == "xk1" else N_LT):
                    ps = pp_ps.tile([128, CH], F32, tag="pp")
                    for ft in range(N_FT):
                        nc.tensor.matmul(
                            ps[:], _R(wd_sb[:, ft, lt * 128:(lt + 1) * 128]), _R(xw[:, ft, :]),
                            start=(ft == 0), stop=(ft == N_FT - 1),
                        )
                    nc.vector.tensor_copy(_R(latc[:, lt, :]), ps[:])
                if which == 0:
                    latk = latc
                else:
                    latv = latc
            if _stage0 == "xk1":
                continue

            if _stage0 in ("lat", "lat1", "lat2", "lat3"):
                continue
            # q projection + rope
            ps = pp_ps.tile([128, CH], F32, tag="pp")
            psr = pp_ps.tile([128, CH], F32, tag="pp")
            for dt in range(N_DT):
                nc.tensor.matmul(ps[:], _R(wq_sb[:, dt, :]), _R(xT[:, dt, :]),
                                 start=(dt == 0), stop=(dt == N_DT - 1))
            for dt in range(N_DT):
                nc.tensor.matmul(psr[:], _R(wqr_sb[:, dt, :]), _R(xT[:, dt, :]),
                                 start=(dt == 0), stop=(dt == N_DT - 1))
            rope(ps, psr, qro[:, c0:c0 + CH], c0)
            if _stage0 == "q1":
                continue

            # k up-projection + rope
            ps = pp_ps.tile([128, CH], F32, tag="pp")
            psr = pp_ps.tile([128, CH], F32, tag="pp")
            for lt in range(N_LT):
                nc.tensor.matmul(ps[:], _R(wku_sb[:, lt, :]), _R(latk[:, lt, :]),
                                 start=(lt == 0), stop=(lt == N_LT - 1))
            for lt in range(N_LT):
                nc.tensor.matmul(psr[:], _R(wkur_sb[:, lt, :]), _R(latk[:, lt, :]),
                                 start=(lt == 0), stop=(lt == N_LT - 1))
            rope(ps, psr, kro[:, c0:c0 + CH], c0)
            if _stage0 == "k1":
                continue

            # v up-projection -> natural layout + ones column
            ps = pp_ps.tile([128, CH], F32, tag="pp")
            for lt in range(N_LT):
                nc.tensor.matmul(ps[:], _R(wvu_sb[:, lt, :]), _R(latv[:, lt, :]),
                                 start=(lt == 0), stop=(lt == N_LT - 1))
            vtmp = vtmp_p.tile([128, CH], F32, tag="vtmp")
            nc.scalar.copy(_R(vtmp[:]), ps[:])
            if _stage0 == "v1":
                continue
            tps = tr_ps.tile([128, 512], F32, tag="trps")
            for st in range(4):
                nc.tensor.transpose(
                    _R(tps[:, st * 128:(st + 1) * 128]),
                    _R(vtmp[:, st * 128:(st + 1) * 128]),
                    _R(idn_sb[:]),
                )
            for st in range(4):
                kt = ci * 4 + st
                for h in range(HPC):
                    off = st * 128 + h * HD
                    nc.scalar.copy(_R(vaug[:, kt, h * (HD + 1):h * (HD + 1) + HD]), tps[:, off:off + HD])

        # ---- attention ----
        _stage = os.environ.get("BIS_STAGE", "full")
        if _stage in ("proj", "proj1", "lat", "lat1", "xk1", "lat2", "lat3", "q1", "k1", "v1"):
            for st in range(N_ST):
                ob = ob_p.tile([128, 512], F32, tag="ob")
                nc.vector.tensor_copy(ob[:], cos_sb[:, 0:512])
                nc.sync.dma_start(out_d[st * 128:(st + 1) * 128, 0:512], ob[:])
        for _rep, qc in ([(r, c) for r in range(_KREPS) for c in range(N_CH)] if _stage in ("attn", "full") else []):
            for h in range(HPC):
                hb = h * HD
                nkt = 4 * (qc + 1)
                av = av_ps.tile([HD + 1, 512], F32, tag="avps")
                for kt in range(nkt):
                    sc = sc_ps.tile([128, 512], F32, tag="scps")
                    nc.tensor.matmul(
                        sc[:], _R(kro[hb:hb + HD, kt * 128:(kt + 1) * 128]),
                        _R(qro[hb:hb + HD, qc * 512:(qc + 1) * 512]),
                        start=True, stop=True,
                    )
                    ex = exp_p.tile([128, 512], F32, tag="exp")
                    nc.scalar.activation(_R(ex[:]), sc[:], mybir.ActivationFunctionType.Exp, scale=float(SCALE))
                    r = kt - 4 * qc
                    if r >= 0:
                        w = 128 * (r + 1)
                        nc.vector.tensor_mul(_R(ex[:, 0:w]), ex[:, 0:w], msk_sb[:, 512 - w:512])
                    nc.tensor.matmul(
                        av[:], _R(vaug[:, kt, h * (HD + 1):(h + 1) * (HD + 1)]), _R(ex[:]),
                        start=(kt == 0), stop=(kt == nkt - 1), skip_group_check=True,
                    )
                onrm = small_p.tile([HD, 512], F32, tag="onrm")
                if os.environ.get("BIS_NONORM"):
                    nc.vector.tensor_copy(onrm[:], av[0:HD, :])
                else:
                    nrm = small_p.tile([128, 512], F32, tag="nrm")
                    nc.vector.reciprocal(nrm[HD:HD + 1, :], av[HD:HD + 1, :])
                    rrow = small_p.tile([1, 512], F32, tag="rrow")
                    nc.sync.dma_start(rrow[:], nrm[HD:HD + 1, :])
                    bcast = small_p.tile([HD, 512], F32, tag="bcast")
                    nc.gpsimd.partition_broadcast(bcast[:], rrow[:])
                    nc.vector.tensor_mul(onrm[:], av[0:HD, :], bcast[:])
                if os.environ.get("BIS_NOSBDMA"):
                    nc.scalar.copy(_R(o_inT[0:HD, qc * 512:(qc + 1) * 512]), onrm[:])
                else:
                    nc.sync.dma_start(_R(o_inT[hb:hb + HD, qc * 512:(qc + 1) * 512]), _R(onrm[:]))

        # ---- partial o-projection ----
        if _stage == "attn":
            for st in range(N_ST):
                ob = ob_p.tile([128, 512], F32, tag="ob")
                nc.vector.tensor_copy(ob[:], o_inT[:, 0:512])
                nc.sync.dma_start(out_d[st * 128:(st + 1) * 128, 0:512], ob[:])
        for _rep, st in ([(r, c) for r in range(_KREPS) for c in range(N_ST)] if _stage == "full" else []):
            for nck in range(2):
                ps = pp_ps.tile([128, 512], F32, tag="pp")
                nc.tensor.matmul(
                    ps[:], _R(o_inT[:, st * 128:(st + 1) * 128]), _R(wo_sb[:, nck * 512:(nck + 1) * 512]),
                    start=True, stop=True,
                )
                ob = ob_p.tile([128, 512], F32, tag="ob")
                nc.vector.tensor_copy(ob[:], ps[:])
                nc.sync.dma_start(out_d[st * 128:(st + 1) * 128, nck * 512:(nck + 1) * 512], ob[:])

    nc.compile()
    return nc


def _get_nc():
    if "nc" not in _CACHE:
        _CACHE["nc"] = _build_nc()
    return _CACHE["nc"]


def _rot_cols(w):
    """Per 64-col head block: [w1, w2] -> [-w2, w1]  (rotate_half on output features)."""
    w = np.asarray(w, np.float32)
    out = np.empty_like(w)
    for h in range(w.shape[1] // HD):
        b = h * HD
        out[:, b:b + 32] = -w[:, b + 32:b + 64]
        out[:, b + 32:b + 64] = w[:, b:b + 32]
    return np.ascontiguousarray(out)


def make_in_maps(x, Wq, Wk, Wv, Wkd, Wvd, Wku, Wvu, Wo):
    x2 = np.ascontiguousarray(np.asarray(x, dtype=np.float32).reshape(S, D))
    cos2, sin2 = _rope_tables()
    msk = _mask_strip()
    idn = np.eye(128, dtype=np.float32)
    common = {
        "x": x2,
        "wk": np.ascontiguousarray(np.asarray(Wk, np.float32)),
        "wkd": np.ascontiguousarray(np.asarray(Wkd, np.float32)),
        "wv": np.ascontiguousarray(np.asarray(Wv, np.float32)),
        "wvd": np.ascontiguousarray(np.asarray(Wvd, np.float32)),
        "cos2": cos2, "sin2": sin2, "mask": msk, "ident": idn,
        "ones16": np.ones((128, N_ST), np.float32),
    }
    Wq = np.asarray(Wq, np.float32)
    Wku = np.asarray(Wku, np.float32)
    Wvu = np.asarray(Wvu, np.float32)
    Wo = np.asarray(Wo, np.float32)
    in_maps = []
    for c in range(N_CORES):
        sl = slice(c * FPC, (c + 1) * FPC)
        in_maps.append(dict(
            common,
            wq=np.ascontiguousarray(Wq[:, sl]),
            wqr=_rot_cols(Wq[:, sl]),
            wkur=_rot_cols(Wku[:, sl]),
            wku=np.ascontiguousarray(Wku[:, sl]),
            wvu=np.ascontiguousarray(Wvu[:, sl]),
            wo=np.ascontiguousarray(Wo[sl, :]),
        ))
    return in_maps


def kernel(x, Wq, Wk, Wv, Wkd, Wvd, Wku, Wvu, Wo):
    nc = _get_nc()
    in_maps = make_in_maps(x, Wq, Wk, Wv, Wkd, Wvd, Wku, Wvu, Wo)
    res = run_bass_kernel_spmd(nc, in_maps, list(range(N_CORES)))
    acc = res.results[0]["out"].astype(np.float32)
    for c in range(1, N_CORES):
        acc = acc + res.results[c]["out"]
    return acc.reshape(B, S, D)
